# revision 6
# baseline (speedup 1.0000x reference)
"""Additive (Bahdanau) attention on 8 TRN2 NeuronCores — v3.

Score s[q,k] = sum_h w_v[h] tanh(qf+kf) ~= sum_j a_j sin(w_j (qf+kf)),
expanded into separable sin/cos feature matmuls.

v3 architecture:
  * kq-concat: qf (Q=128 cols) is appended to kf (K=1024 cols) in one
    [128, 2, 1152] tile, so the q-side features ride through the SAME
    reflection-fold chains and Sin passes as the k-side — the entire
    per-frequency q-side fold pipeline of v2 is gone.
  * all elementwise on DVE (GpSimd streaming turned out to lock the shared
    SBUF port and halve DVE throughput), kq casts on ScalarE (PSUM-near),
    softmax row sums via the Exp accum_out.
  * per-chunk score matmuls flow as features land; tickler transposes keep
    the PE HAM clock at 2.4 GHz through the feature ramp.
  * a doubled base is scheduled last so its DVE product+chunk tail hides
    the ~2.7us exp ACT-table load.
  * config-selectable frequency sets (4 bases + 2 doubles, or 3+2).
"""

import numpy as np

B, Q, K, D, H = 8, 128, 1024, 256, 256
KQ = K + Q
NEG_BIG = -60000.0
PI = float(np.pi)
XMAX = 5.5

CFG4 = dict(
    wb=[0.2578618179453934, 0.7767074933031182, 1.2843070746734042,
        1.8410383299779165],
    a=[1.2425067487497459, 0.3336529012015775, 0.1379759654293557,
       0.06966796134575332, 0.02996498493319701, 0.007819070789205603],
    dbl=[2, 3],
    act_order=[0, 2, 1, 3],
)
CFG3 = dict(
    wb=[0.2576843694572776, 0.8143873794549281, 1.2676205020377622],
    a=[1.2624871436620988, 0.34991764369874145, 0.0655282927999729,
       0.1037290720504511, 0.03993465739431127],
    dbl=[1, 2],
    act_order=[0, 1, 2],
)
# double 2w1 + quad 4w1 both derive from the middle base: no DVE product
# work remains after the last Sin (tail = g + chunks + exp only)
CFG5 = dict(
    wb=[0.20246930773159078, 0.6310887691730037, 1.8460734157868008],
    a=[1.2115443342732137, 0.4533555532233307, 0.057624418129119066,
       0.18918115397225269, 0.03351562009001214],
    dbl=[1],
    quad=1,
    act_order=[0, 1, 2],
)
CONFIG = CFG3

_CACHE = {}


def _nfolds(w):
    u0max = w * XMAX + PI / 2
    if u0max <= PI - 0.05:
        return 0
    if u0max <= 2 * PI - 0.1:
        return 1
    return 2


def _build_bass(cfg):
    import concourse.bass as bass
    import concourse.tile as tile
    from concourse import mybir
    from concourse.masks import make_identity
    from concourse.bass import broadcast_tensor_aps
    from contextlib import ExitStack

    F32 = mybir.dt.float32
    F16 = mybir.dt.float16
    AF = mybir.ActivationFunctionType
    OP = mybir.AluOpType

    WBc = cfg["wb"]
    Ac = cfg["a"]
    DBL = cfg["dbl"]
    ORDER = cfg["act_order"]
    nb = len(WBc)
    nd = len(DBL)
    nst = nb + nd + (1 if cfg.get("quad") else 0)
    n_folded = sum(1 for w in WBc if _nfolds(w) > 0)

    nc = bass.Bass()
    abs_patch = []

    # host pre-arranged [128, ...] layouts: every DMA line is a contiguous
    # multi-KB run (small-packet DMA costs ~360ns/packet/engine)
    NSM = Q + H + nst * 2 + 1
    kT_ext = nc.declare_dram_parameter("kT4", [128, 2, 2, 512], F16, isOutput=False)
    vals_ext = nc.declare_dram_parameter("vals3", [128, 8, D], F16, isOutput=False)
    wkT_ext = nc.declare_dram_parameter("wkTp", [128, 2, H], F16, isOutput=False)
    small_ext = nc.declare_dram_parameter("small", [128, 2, NSM], F16, isOutput=False)
    mask_ext = nc.declare_dram_parameter("mask", [1, K], F16, isOutput=False)
    out_ext = nc.declare_dram_parameter("out", [Q, D], F32, isOutput=True)

    def ts_abs(out, in_, s1, s2, op0, op1, patch):
        if op1 is None:
            i = nc.vector.tensor_scalar(out, in_, s1, s2, op0=op0)
        else:
            i = nc.vector.tensor_scalar(out, in_, s1, s2, op0=op0, op1=op1)
        abs_patch.append((i.ins.name, patch))
        return i

    with tile.TileContext(nc) as tc, ExitStack() as ctx:
        persist = ctx.enter_context(tc.tile_pool(name="persist", bufs=1))
        scores_ps = ctx.enter_context(tc.tile_pool(name="scores_ps", bufs=1, space="PSUM"))
        argk_pool = ctx.enter_context(
            tc.tile_pool(name="argk_pool", bufs=2))
        tick_deps = []

        # ---- persistent SBUF tiles ----
        kT_sb = persist.tile([128, 2, 2, 512], F16)   # [c, t, k-in-half]
        wkT_sb = persist.tile([128, 2, H], F16)
        small_sb = persist.tile([128, 2, NSM], F16)
        qT_sb = small_sb[:, :, 0:Q]
        wqT_sb = small_sb[:, :, Q:Q + H]
        m2_off = Q + H

        def m2_slice(j, gs):
            return small_sb[:, :, m2_off + 2 * j + gs:m2_off + 2 * j + gs + 1]

        wv2_sb = small_sb[:, :, m2_off + 2 * nst:m2_off + 2 * nst + 1]
        val_sb = persist.tile([128, 8, D], F16)
        mask_sb = persist.tile([1, K], F16)
        ones_sb = persist.tile([1, 128], F16)
        onesk_sb = persist.tile([128, 512], F16)
        ident = persist.tile([128, 128], F16)
        kqk_sb = persist.tile([128, 2, K], F16)     # kf features
        kqq_sb = persist.tile([128, 2, Q], F16)     # qf features (separate
        # tile: whole-tile WAR granularity would stall these casts behind
        # F0's reads of the k-half otherwise)
        zkq_sb = persist.tile([128, 2, KQ], F16)
        F_t = [persist.tile([128, 2, 2, KQ], F16, name=f"F{i}") for i in range(nb)]
        P_t = [persist.tile([128, 2, 2, K], F16, name=f"P{d}") for d in range(nd)]
        scd = [persist.tile([128, 2, 2, Q], F16, name=f"scd{d}") for d in range(nd)]
        s2tmp = persist.tile([128, 2, Q], F16)
        g_sb = persist.tile([128, nst, 2, 2, Q], F16)
        u1_sb = persist.tile([128, 2, Q], F16)
        u2_sb = persist.tile([128, 2, Q], F16)
        gone_sb = persist.tile([128, 2, Q], F16)
        E_q = [persist.tile([128, K // 4], F16, name=f"E{i}") for i in range(4)]
        ET_sb = persist.tile([128, 8, 128], F16)
        out_sb = persist.tile([Q, D], F32)
        pihalf = persist.tile([128, 1], F32)
        zero_b = persist.tile([128, 1], F32)
        shift_sb = persist.tile([128, 1], F32)
        dummy = persist.tile([128, 1], F32)
        rs_q = [persist.tile([128, 1], F32, name=f"rs{i}") for i in range(4)]
        rowsum = persist.tile([128, 1], F32)
        rs_t = persist.tile([128, 1], F32)
        rinv = persist.tile([128, 1], F32)

        # ---- DMA: wkT + kT column-halves first (each delivers BOTH t-halves
        # of 512 k-columns, so kf matmuls start after one 256KB transfer);
        # small tensors + vals issue from the idle GpSimd queue in parallel ----
        nc.scalar.dma_start(out=wkT_sb, in_=wkT_ext[:, :, :])
        nc.sync.dma_start(out=kT_sb[:, 0, :, :], in_=kT_ext[:, 0, :, :])
        nc.sync.dma_start(out=kT_sb[:, 1, :, :], in_=kT_ext[:, 1, :, :])
        nc.scalar.dma_start(out=small_sb, in_=small_ext[:, :, :])
        nc.gpsimd.dma_start(out=mask_sb, in_=mask_ext[:, :])
        nc.vector.memset(ones_sb, 1.0)
        nc.vector.memset(onesk_sb, 1.0)
        nc.vector.memset(pihalf, PI / 2)
        nc.vector.memset(zero_b, 0.0)
        nc.vector.memset(shift_sb, -5.0)
        make_identity(nc, ident)
        nc.scalar.activation(dummy, pihalf, AF.Sin, scale=0.1)

        scores_a = scores_ps.tile([128, K // 2], F32, tag="sca")
        scores_b = scores_ps.tile([128, K // 2], F32, tag="scb")
        scores_c = [scores_a, scores_b]

        tick_ctx = ExitStack()
        tk_pool = tick_ctx.enter_context(
            tc.tile_pool(name="tk_ps", bufs=1, space="PSUM"))
        tk = tk_pool.tile([128, 128], F16)

        # ---- projections (kf first, c-outer: each kT column-half unblocks
        # its matmuls); per-(ht,c) kq casts on ScalarE ----
        setup_ctx = ExitStack()
        kf_ps_pool = setup_ctx.enter_context(
            tc.tile_pool(name="kf_ps", bufs=1, space="PSUM"))
        qf_ps_pool = setup_ctx.enter_context(
            tc.tile_pool(name="qf_ps", bufs=1, space="PSUM"))
        kf_ps = kf_ps_pool.tile([128, 2, K], F32, tag="kfp")
        for c in range(2):
            csl = slice(c * 512, (c + 1) * 512)
            for ht in range(2):
                hsl = slice(ht * 128, (ht + 1) * 128)
                nc.tensor.matmul(kf_ps[:, ht, csl], wkT_sb[:, 0, hsl],
                                 kT_sb[:, c, 0, :], start=True, stop=False)
                nc.tensor.matmul(kf_ps[:, ht, csl], wkT_sb[:, 1, hsl],
                                 kT_sb[:, c, 1, :], start=False, stop=True)
            # parallel casts: ht0 on ScalarE, ht1 on DVE
            nc.scalar.copy(kqk_sb[:, 0, c * 512:(c + 1) * 512],
                           kf_ps[:, 0, csl])
            nc.vector.tensor_copy(kqk_sb[:, 1, c * 512:(c + 1) * 512],
                                  kf_ps[:, 1, csl])

        ps_q = qf_ps_pool.tile([128, 2, Q], F32, tag="psq")
        for ht in range(2):
            hsl = slice(ht * 128, (ht + 1) * 128)
            nc.tensor.matmul(ps_q[:, ht, :], wqT_sb[:, 0, hsl], qT_sb[:, 0, :],
                             start=True, stop=False)
            nc.tensor.matmul(ps_q[:, ht, :], wqT_sb[:, 1, hsl], qT_sb[:, 1, :],
                             start=False, stop=True)
            nc.vector.tensor_copy(kqq_sb[:, ht, :], ps_q[:, ht, :])
        setup_ctx.close()

        # mask opener after the projections (mask DMA is on the slow queue)
        for c in range(2):
            csl = slice(c * 512, (c + 1) * 512)
            nc.tensor.matmul(scores_c[c], ones_sb, mask_sb[:, csl],
                             start=True, stop=False)

        # ---- DVE: zkq, then per-base fold chains in ACT order ----
        ts_abs(zkq_sb[:, :, 0:K], kqk_sb, 0.0, None, op0=OP.max, op1=None,
               patch="op0")
        ts_abs(zkq_sb[:, :, K:KQ], kqq_sb, 0.0, None, op0=OP.max, op1=None,
               patch="op0")
        # vals (512KB, needed only at the AV tail) issues only after zkq
        # exists, so it doesn't steal inbound DMA bandwidth from the
        # critical kT/weights path
        dma_gate = persist.tile([128, 1], F16)
        nc.gpsimd.tensor_copy(dma_gate, zkq_sb[:, 0, 0:1])
        nc.gpsimd.dma_start(out=val_sb, in_=vals_ext[:, :, :])

        def emit_kfolds(i):
            # every pass reads a FULL tile (sliced inputs drop the DVE to
            # 2x mode: measured 1353ns vs 753ns for the same element count)
            w = WBc[i]
            nf = _nfolds(w)
            if nf == 0:
                return None
            s_w = PI / (2 * w)
            C0 = 2 * PI if nf == 2 else PI
            asin = argk_pool.tile([128, 2, KQ], F16, tag="ka")
            a2 = argk_pool.tile([128, 2, 2, KQ], F16, tag="kb")
            a3 = argk_pool.tile([128, 2, 2, KQ], F16, tag="kc")
            a4 = argk_pool.tile([128, 2, 2, KQ], F16, tag="kd")
            ts_abs(asin[:, :, 0:K], kqk_sb, s_w, 0.0,
                   op0=OP.subtract, op1=OP.max, patch="op1")
            ts_abs(asin[:, :, K:KQ], kqq_sb, s_w, 0.0,
                   op0=OP.subtract, op1=OP.max, patch="op1")
            tick_deps.append(asin[:, 0, 0:128])
            nc.vector.tensor_scalar(a2[:, 0, :, :], asin, w, C0,
                                    op0=OP.mult, op1=OP.subtract)
            nc.vector.tensor_scalar(a2[:, 1, :, :], zkq_sb, w, C0,
                                    op0=OP.mult, op1=OP.subtract)
            tick_deps.append(a2[:, 1, 0, 0:128])
            if nf == 1:
                ts_abs(a3, a2, 0.0, PI / 2, op0=OP.max, op1=OP.subtract,
                       patch="op0")
                return a3
            ts_abs(a3, a2, 0.0, PI, op0=OP.max, op1=OP.subtract, patch="op0")
            tick_deps.append(a3[:, 1, 0, 0:128])
            ts_abs(a4, a3, 0.0, PI / 2, op0=OP.max, op1=OP.subtract,
                   patch="op0")
            return a4

        def emit_ksin(i, argk):
            if argk is None:
                w0_ = WBc[i]
                nc.scalar.activation(F_t[i][:, 0, :, 0:K], kqk_sb, AF.Sin,
                                     bias=zero_b, scale=w0_)
                nc.scalar.activation(F_t[i][:, 1, :, 0:K], kqk_sb, AF.Sin,
                                     bias=pihalf, scale=w0_)
                nc.scalar.activation(F_t[i][:, 0, :, K:KQ], kqq_sb, AF.Sin,
                                     bias=zero_b, scale=w0_)
                nc.scalar.activation(F_t[i][:, 1, :, K:KQ], kqq_sb, AF.Sin,
                                     bias=pihalf, scale=w0_)
            else:
                nc.scalar.activation(F_t[i], argk, AF.Sin, bias=zero_b,
                                     scale=1.0)

        def emit_g(j):
            # stationary build: g[:, j, gs] = F_j q-cols (side gs) * coef*wv
            for gs in range(2):
                sa, sb_ = broadcast_tensor_aps(F_t[j][:, gs, :, K:KQ],
                                               m2_slice(j, gs))
                nc.vector.tensor_tensor(out=g_sb[:, j, gs, :, :], in0=sa,
                                        in1=sb_, op=OP.mult)

        def emit_scd(d, bi):
            # scd[d][:,0] = sin*cos (q-cols); scd[d][:,1] = 1-2 sin^2
            nc.vector.tensor_tensor(out=scd[d][:, 0, :, :],
                                    in0=F_t[bi][:, 0, :, K:KQ],
                                    in1=F_t[bi][:, 1, :, K:KQ], op=OP.mult)
            nc.vector.tensor_tensor(out=s2tmp, in0=F_t[bi][:, 0, :, K:KQ],
                                    in1=F_t[bi][:, 0, :, K:KQ], op=OP.mult)
            nc.vector.tensor_scalar(scd[d][:, 1, :, :], s2tmp, -2.0, 1.0,
                                    op0=OP.mult, op1=OP.add)
            for kind in range(2):
                sa, sb_ = broadcast_tensor_aps(scd[d][:, kind, :, :],
                                               m2_slice(nb + d, kind))
                nc.vector.tensor_tensor(out=g_sb[:, nb + d, kind, :, :],
                                        in0=sa, in1=sb_, op=OP.mult)

        def emit_P(d, bi):
            # Pc first: the d-chunk gs0 matmuls consume it
            nc.vector.tensor_tensor(out=P_t[d][:, 1, :, :],
                                    in0=F_t[bi][:, 0, :, 0:K],
                                    in1=F_t[bi][:, 0, :, 0:K], op=OP.mult)
            nc.vector.tensor_tensor(out=P_t[d][:, 0, :, :],
                                    in0=F_t[bi][:, 0, :, 0:K],
                                    in1=F_t[bi][:, 1, :, 0:K], op=OP.mult)

        # ACT: sins in ORDER;  DVE: folds + per-base g/scd/P as sins land
        args = {}
        order_folded = [i for i in ORDER if _nfolds(WBc[i]) > 0]
        order_direct = [i for i in ORDER if _nfolds(WBc[i]) == 0]
        # fold chains first-two up-front, rest interleaved after g of direct
        for i in order_folded[:1]:
            args[i] = emit_kfolds(i)
        for i in order_direct:
            emit_ksin(i, None)
        for i in order_folded[:1]:
            emit_ksin(i, args[i])
        for i in order_direct:
            emit_g(i)
        prev = order_folded[0]
        for i in order_folded[1:]:
            args[i] = emit_kfolds(i)
            emit_ksin(i, args[i])
            # post-sin DVE work of the PREVIOUS folded base
            emit_g(prev)
            if prev in DBL:
                d = DBL.index(prev)
                emit_scd(d, prev)
                emit_P(d, prev)
            prev = i
        if cfg.get("quad"):
            # quad features from the double of the middle base: sin4q/4 =
            # sc*c2, cos4q = 1-8 sc^2; k-side Ps^2 / Ps*Pc products.
            squad = persist.tile([128, 2, 2, Q], F16)
            g_x = persist.tile([128, 2, Q], F16)
            P2q = persist.tile([128, 2, 2, K], F16)
            qslot = nb + nd
            nc.vector.tensor_tensor(out=squad[:, 0, :, :],
                                    in0=scd[0][:, 0, :, :],
                                    in1=scd[0][:, 1, :, :], op=OP.mult)
            nc.vector.tensor_tensor(out=s2tmp, in0=scd[0][:, 0, :, :],
                                    in1=scd[0][:, 0, :, :], op=OP.mult)
            nc.vector.tensor_scalar(squad[:, 1, :, :], s2tmp, -8.0, 1.0,
                                    op0=OP.mult, op1=OP.add)
            for kind in range(2):
                sa, sb_ = broadcast_tensor_aps(squad[:, kind, :, :],
                                               m2_slice(qslot, kind))
                nc.vector.tensor_tensor(out=g_sb[:, qslot, kind, :, :],
                                        in0=sa, in1=sb_, op=OP.mult)
            nc.vector.tensor_scalar_mul(g_x, g_sb[:, qslot, 1, :, :], -0.5)
            nc.vector.tensor_tensor(out=P2q[:, 1, :, :],
                                    in0=P_t[0][:, 0, :, :],
                                    in1=P_t[0][:, 0, :, :], op=OP.mult)
            nc.vector.tensor_tensor(out=P2q[:, 0, :, :],
                                    in0=P_t[0][:, 0, :, :],
                                    in1=P_t[0][:, 1, :, :], op=OP.mult)
            nc.vector.tensor_scalar_mul(u1_sb, scd[0][:, 0, :, :],
                                        float(2 * Ac[nb]))
            nc.vector.scalar_tensor_tensor(u2_sb, squad[:, 0, :, :],
                                           float(4 * Ac[nb + 1]), u1_sb,
                                           op0=OP.mult, op1=OP.add)
            ga, gb = broadcast_tensor_aps(u2_sb[:, :, :], wv2_sb[:, :, :])
            nc.vector.tensor_tensor(out=gone_sb, in0=ga, in1=gb, op=OP.mult)
            emit_g(prev)
        else:
            emit_g(prev)
            if prev in DBL:
                d = DBL.index(prev)
                emit_scd(d, prev)
            # ones-chunk combiners (need all scd)
            nc.vector.tensor_scalar_mul(u1_sb, scd[0][:, 0, :, :],
                                        float(2 * Ac[nb + 0]))
            nc.vector.scalar_tensor_tensor(u2_sb, scd[1][:, 0, :, :],
                                           float(2 * Ac[nb + 1]), u1_sb,
                                           op0=OP.mult, op1=OP.add)
            ga, gb = broadcast_tensor_aps(u2_sb[:, :, :], wv2_sb[:, :, :])
            nc.vector.tensor_tensor(out=gone_sb, in0=ga, in1=gb, op=OP.mult)
            if prev in DBL:
                emit_P(DBL.index(prev), prev)

        # ---- HAM ticklers bridging the feature ramp ----
        nc.tensor.transpose(tk, kqk_sb[:, 1, 0:128], ident)
        for dep in tick_deps[:6]:
            nc.tensor.transpose(tk, dep, ident)

        # ---- score chunks ----
        def emit_chunk(jslot, Fm, last=False):
            for gs in range(2):
                for t in range(2):
                    for c in range(2):
                        csl = slice(c * 512, (c + 1) * 512)
                        stop = last and gs == 1 and t == 1
                        nc.tensor.matmul(scores_c[c], g_sb[:, jslot, gs, t, :],
                                         Fm[:, 1 - gs, t, csl],
                                         start=False, stop=stop)

        chunk_order = []
        for i in ORDER:
            chunk_order.append(("b", i))
            if i in DBL:
                chunk_order.append(("d", DBL.index(i)))
        if cfg.get("quad"):
            # [b0, b1, d0, quad, ones, b2]: the last base carries no products
            chunk_order.insert(3, ("q4", None))
            chunk_order.insert(4, ("ones", None))
        else:
            # ones inserted before the final double
            chunk_order.insert(len(chunk_order) - 1, ("ones", None))
        for idx, (kind, v) in enumerate(chunk_order):
            last = idx == len(chunk_order) - 1
            if kind == "b":
                emit_chunk(v, F_t[v], last=last)
            elif kind == "d":
                emit_chunk(nb + v, P_t[v], last=last)
            elif kind == "q4":
                for stat, mov in ((g_sb[:, nb + nd, 0, :, :], P2q[:, 1, :, :]),
                                  (g_sb[:, nb + nd, 1, :, :], P2q[:, 0, :, :]),
                                  (g_x, P_t[0][:, 0, :, :])):
                    for t in range(2):
                        for c in range(2):
                            csl = slice(c * 512, (c + 1) * 512)
                            nc.tensor.matmul(scores_c[c], stat[:, t, :],
                                             mov[:, t, csl],
                                             start=False, stop=False)
            else:
                for t in range(2):
                    for c in range(2):
                        nc.tensor.matmul(scores_c[c], gone_sb[:, t, :],
                                         onesk_sb, start=False, stop=False)
        tick_ctx.close()

        # ---- softmax: exp(s-5), row sums on DVE (idle by then) ----
        for qtr in range(4):
            sc = scores_c[qtr // 2]
            off = (qtr % 2) * 256
            nc.scalar.activation(E_q[qtr], sc[:, off:off + 256], AF.Exp,
                                 bias=shift_sb, scale=1.0)
            nc.vector.tensor_reduce(rs_q[qtr], E_q[qtr],
                                    axis=mybir.AxisListType.X,
                                    op=mybir.AluOpType.add)
        nc.vector.tensor_add(rowsum, rs_q[0], rs_q[1])
        nc.vector.tensor_add(rs_t, rs_q[2], rs_q[3])
        nc.vector.tensor_add(rowsum, rowsum, rs_t)
        nc.vector.reciprocal(rinv, rowsum)

        # ---- attn @ values ----
        with ExitStack() as tail_ctx:
            tp_ps = tail_ctx.enter_context(
                tc.tile_pool(name="tp_ps", bufs=2, space="PSUM"))
            av_ps = tail_ctx.enter_context(
                tc.tile_pool(name="av_ps", bufs=1, space="PSUM"))
            for kt in range(8):
                E_src = E_q[kt // 2]
                off = (kt % 2) * 128
                tp = tp_ps.tile([128, 128], F16, tag="tp")
                nc.tensor.transpose(tp, E_src[:, off:off + 128], ident)
                nc.vector.tensor_copy(ET_sb[:, kt, :], tp)
            ps_av = av_ps.tile([Q, D], F32)
            for kt in range(8):
                nc.tensor.matmul(ps_av, ET_sb[:, kt, :], val_sb[:, kt, :],
                                 start=(kt == 0), stop=(kt == 7))
            nc.vector.tensor_scalar_mul(out_sb[:, 0:128], ps_av[:, 0:128], rinv)
            nc.sync.dma_start(out=out_ext[:, 0:128], in_=out_sb[:, 0:128])
            nc.vector.tensor_scalar_mul(out_sb[:, 128:256], ps_av[:, 128:256], rinv)
        nc.sync.dma_start(out=out_ext[:, 128:256], in_=out_sb[:, 128:256])

    _patch_bir(nc, abs_patch)
    return nc


def _patch_bir(nc, abs_patch):
    import json

    d = json.loads(nc.to_json_bytes())
    patch_map = dict(abs_patch)
    k = [0]
    self_drop = {"Activation": "Activation", "DVE": "DVE", "Pool": "Pool"}
    compute_ops = {"Activation", "TensorScalarPtr", "TensorScalar", "TensorTensor",
                   "TensorCopy", "TensorReduce", "Reciprocal", "Memset"}
    n_abs = 0
    for fn in d["functions"]:
        for blk in fn["blocks"]:
            out = []
            for inst in blk["instructions"]:
                slot = patch_map.get(inst.get("name"))
                if slot is not None:
                    inst[slot] = "abs"
                    n_abs += 1
                si = inst.get("sync_info") or {}
                ow = si.get("on_wait") or []
                op = inst.get("opcode")
                eng = inst.get("engine")
                if len(ow) > 1 and op != "EventSemaphore":
                    if op in compute_ops and eng in self_drop:
                        pref = self_drop[eng] + "_"
                        ow = [w for w in ow
                              if not str(w.get("ant_name", "")).startswith(pref)]
                    while len(ow) > 1:
                        w = ow.pop(0)
                        k[0] += 1
                        out.append({
                            "debug": inst.get("debug", 0), "engine": eng,
                            "ins": [], "name": f"WSplit-{k[0]}",
                            "opcode": "EventSemaphore", "outs": [],
                            "sync_info": {"on_update": [], "on_wait": [w]},
                        })
                    si["on_wait"] = ow
                out.append(inst)
            blk["instructions"] = out
    assert n_abs == len(abs_patch), (n_abs, len(abs_patch))
    patched = json.dumps(d).encode()
    nc.to_json_bytes = lambda: patched


def _get_nc():
    if "nc" not in _CACHE:
        _CACHE["nc"] = _build_bass(CONFIG)
    return _CACHE["nc"]


def _host_prep(queries, keys, values, W_q, W_k, w_v, valid_lens):
    cfg = CONFIG
    nb = len(cfg["wb"])
    nd = len(cfg["dbl"])
    nst = nb + nd + (1 if cfg.get("quad") else 0)
    Ac = cfg["a"]

    queries = np.asarray(queries, dtype=np.float32)
    keys = np.asarray(keys, dtype=np.float32)
    values = np.asarray(values, dtype=np.float32)
    W_q = np.asarray(W_q, dtype=np.float32)
    W_k = np.asarray(W_k, dtype=np.float32)
    w_v = np.asarray(w_v, dtype=np.float32)
    valid = np.asarray(valid_lens).astype(np.int64)

    def part2(x):
        # [2*128, N] -> [128, 2, N]
        return x.reshape(2, 128, -1).transpose(1, 0, 2)

    wqT = part2(W_q.T.astype(np.float16))            # [128, 2, H]
    wkT = part2(W_k.T.astype(np.float16))
    wv2 = w_v.reshape(2, 128).T                      # (p, t)
    coef = np.zeros((nst, 2), np.float32)
    for j in range(nb):
        coef[j] = (Ac[j], Ac[j])
    for d in range(nd):
        # scd kind0 = sin*cos = sin(2w)/2 -> Pc-pair coef -4a; kind1 = cos2w -> +2a
        coef[nb + d] = (-4 * Ac[nb + d], 2 * Ac[nb + d])
    if cfg.get("quad"):
        # squad kind0 = sin4q/4 -> -8a*4; kind1 = cos4q -> -8a
        coef[nb + nd] = (-32 * Ac[nb + nd], -8 * Ac[nb + nd])
    # M2 block [p, t, j*2+gs]
    M2t = (wv2[:, :, None, None] * coef[None, None, :, :]).reshape(128, 2, -1)
    wv2_t = wv2[:, :, None]

    kidx = np.arange(K)
    in_maps = []
    for b in range(B):
        mask = np.where(kidx < valid[b], np.float16(0.0), np.float16(NEG_BIG))
        qTc = part2(queries[b].T.astype(np.float16))     # [128, 2, Q]
        small = np.concatenate(
            [qTc, wqT, M2t.astype(np.float16),
             wv2_t.astype(np.float16)], axis=2)
        in_maps.append({
            "kT4": np.ascontiguousarray(part2(keys[b].T.astype(np.float16))
                .reshape(128, 2, 2, 512).transpose(0, 2, 1, 3)),
            "vals3": np.ascontiguousarray(
                values[b].astype(np.float16).reshape(8, 128, D)
                .transpose(1, 0, 2)),
            "wkTp": np.ascontiguousarray(wkT),
            "small": np.ascontiguousarray(small),
            "mask": np.ascontiguousarray(mask.reshape(1, K)),
        })
    return in_maps, valid, values


def _run(inputs, trace=False, **kw):
    from concourse.bass_utils import run_bass_kernel_spmd

    nc = _get_nc()
    in_maps, valid, values = _host_prep(**inputs)
    res = run_bass_kernel_spmd(nc, in_maps, list(range(B)), trace=trace, **kw)
    out = np.stack([np.asarray(res.results[i]["out"], dtype=np.float32)
                    for i in range(B)])
    for b in range(B):
        if valid[b] == 0:
            out[b] = np.broadcast_to(values[b].mean(axis=0), (Q, D))
    return out, res


def kernel(**inputs):
    out, _ = _run(inputs, trace=False)
    return out


# revision 7
# speedup vs baseline: 1.0337x; 1.0337x over previous
"""Additive (Bahdanau) attention on 8 TRN2 NeuronCores — v3.

Score s[q,k] = sum_h w_v[h] tanh(qf+kf) ~= sum_j a_j sin(w_j (qf+kf)),
expanded into separable sin/cos feature matmuls.

v3 architecture:
  * kq-concat: qf (Q=128 cols) is appended to kf (K=1024 cols) in one
    [128, 2, 1152] tile, so the q-side features ride through the SAME
    reflection-fold chains and Sin passes as the k-side — the entire
    per-frequency q-side fold pipeline of v2 is gone.
  * all elementwise on DVE (GpSimd streaming turned out to lock the shared
    SBUF port and halve DVE throughput), kq casts on ScalarE (PSUM-near),
    softmax row sums via the Exp accum_out.
  * per-chunk score matmuls flow as features land; tickler transposes keep
    the PE HAM clock at 2.4 GHz through the feature ramp.
  * a doubled base is scheduled last so its DVE product+chunk tail hides
    the ~2.7us exp ACT-table load.
  * config-selectable frequency sets (4 bases + 2 doubles, or 3+2).
"""

import numpy as np

B, Q, K, D, H = 8, 128, 1024, 256, 256
KQ = K + Q
NEG_BIG = -60000.0
PI = float(np.pi)
XMAX = 5.5

CFG4 = dict(
    wb=[0.2578618179453934, 0.7767074933031182, 1.2843070746734042,
        1.8410383299779165],
    a=[1.2425067487497459, 0.3336529012015775, 0.1379759654293557,
       0.06966796134575332, 0.02996498493319701, 0.007819070789205603],
    dbl=[2, 3],
    act_order=[0, 2, 1, 3],
)
CFG3 = dict(
    wb=[0.2576843694572776, 0.8143873794549281, 1.2676205020377622],
    a=[1.2624871436620988, 0.34991764369874145, 0.0655282927999729,
       0.1037290720504511, 0.03993465739431127],
    dbl=[1, 2],
    act_order=[0, 1, 2],
)
# double 2w1 + quad 4w1 both derive from the middle base: no DVE product
# work remains after the last Sin (tail = g + chunks + exp only)
CFG5 = dict(
    wb=[0.20246930773159078, 0.6310887691730037, 1.8460734157868008],
    a=[1.2115443342732137, 0.4533555532233307, 0.057624418129119066,
       0.18918115397225269, 0.03351562009001214],
    dbl=[1],
    quad=1,
    act_order=[0, 1, 2],
)
CONFIG = CFG3

_CACHE = {}


def _nfolds(w):
    u0max = w * XMAX + PI / 2
    if u0max <= PI - 0.05:
        return 0
    if u0max <= 2 * PI - 0.1:
        return 1
    return 2


def _build_bass(cfg):
    import concourse.bass as bass
    import concourse.tile as tile
    from concourse import mybir
    from concourse.masks import make_identity
    from concourse.bass import broadcast_tensor_aps
    from contextlib import ExitStack

    F32 = mybir.dt.float32
    F16 = mybir.dt.float16
    AF = mybir.ActivationFunctionType
    OP = mybir.AluOpType

    WBc = cfg["wb"]
    Ac = cfg["a"]
    DBL = cfg["dbl"]
    ORDER = cfg["act_order"]
    nb = len(WBc)
    nd = len(DBL)
    nst = nb + nd + (1 if cfg.get("quad") else 0)
    n_folded = sum(1 for w in WBc if _nfolds(w) > 0)

    nc = bass.Bass()
    abs_patch = []

    # host pre-arranged [128, ...] layouts: every DMA line is a contiguous
    # multi-KB run (small-packet DMA costs ~360ns/packet/engine)
    NSM = nst * 2 + 1
    kT_ext = nc.declare_dram_parameter("kT4", [128, 2, 2, 512], F16, isOutput=False)
    vals_ext = nc.declare_dram_parameter("vals3", [128, 8, D], F16, isOutput=False)
    wkT_ext = nc.declare_dram_parameter("wkTp", [128, 2, H], F16, isOutput=False)
    qw_ext = nc.declare_dram_parameter("qw", [128, 2, Q + H], F16, isOutput=False)
    small_ext = nc.declare_dram_parameter("small", [128, 2, NSM], F16, isOutput=False)
    mask_ext = nc.declare_dram_parameter("mask", [1, K], F16, isOutput=False)
    out_ext = nc.declare_dram_parameter("out", [Q, D], F32, isOutput=True)

    def ts_abs(out, in_, s1, s2, op0, op1, patch):
        if op1 is None:
            i = nc.vector.tensor_scalar(out, in_, s1, s2, op0=op0)
        else:
            i = nc.vector.tensor_scalar(out, in_, s1, s2, op0=op0, op1=op1)
        abs_patch.append((i.ins.name, patch))
        return i

    with tile.TileContext(nc) as tc, ExitStack() as ctx:
        persist = ctx.enter_context(tc.tile_pool(name="persist", bufs=1))
        scores_ps = ctx.enter_context(tc.tile_pool(name="scores_ps", bufs=1, space="PSUM"))
        argk_pool = ctx.enter_context(
            tc.tile_pool(name="argk_pool", bufs=2))
        tick_deps = []

        # ---- persistent SBUF tiles ----
        kT_sb = persist.tile([128, 2, 2, 512], F16)   # [c, t, k-in-half]
        wkT_sb = persist.tile([128, 2, H], F16)
        qw_sb = persist.tile([128, 2, Q + H], F16)
        small_sb = persist.tile([128, 2, NSM], F16)
        qT_sb = qw_sb[:, :, 0:Q]
        wqT_sb = qw_sb[:, :, Q:Q + H]
        m2_off = 0

        def m2_slice(j, gs):
            return small_sb[:, :, m2_off + 2 * j + gs:m2_off + 2 * j + gs + 1]

        wv2_sb = small_sb[:, :, m2_off + 2 * nst:m2_off + 2 * nst + 1]
        val_sb = persist.tile([128, 8, D], F16)
        mask_sb = persist.tile([1, K], F16)
        ones_sb = persist.tile([1, 128], F16)
        onesk_sb = persist.tile([128, 512], F16)
        ident = persist.tile([128, 128], F16)
        kqk_sb = persist.tile([128, 2, K], F16)     # kf features
        kqq_sb = persist.tile([128, 2, Q], F16)     # qf features (separate
        # tile: whole-tile WAR granularity would stall these casts behind
        # F0's reads of the k-half otherwise)
        zkq_sb = persist.tile([128, 2, KQ], F16)
        F_t = [persist.tile([128, 2, 2, KQ], F16, name=f"F{i}") for i in range(nb)]
        P_t = [persist.tile([128, 2, 2, K], F16, name=f"P{d}") for d in range(nd)]
        scd = [persist.tile([128, 2, 2, Q], F16, name=f"scd{d}") for d in range(nd)]
        s2tmp = persist.tile([128, 2, Q], F16)
        g_sb = persist.tile([128, nst, 2, 2, Q], F16)
        u1_sb = persist.tile([128, 2, Q], F16)
        u2_sb = persist.tile([128, 2, Q], F16)
        gone_sb = persist.tile([128, 2, Q], F16)
        E_q = [persist.tile([128, K // 4], F16, name=f"E{i}") for i in range(4)]
        ET_sb = persist.tile([128, 8, 128], F16)
        out_sb = persist.tile([Q, D], F32)
        pihalf = persist.tile([128, 1], F32)
        zero_b = persist.tile([128, 1], F32)
        shift_sb = persist.tile([128, 1], F32)
        dummy = persist.tile([128, 1], F32)
        rs_q = [persist.tile([128, 1], F32, name=f"rs{i}") for i in range(4)]
        rowsum = persist.tile([128, 1], F32)
        rs_t = persist.tile([128, 1], F32)
        rinv = persist.tile([128, 1], F32)

        # ---- DMA: wkT + kT column-halves first (each delivers BOTH t-halves
        # of 512 k-columns, so kf matmuls start after one 256KB transfer);
        # small tensors + vals issue from the idle GpSimd queue in parallel ----
        nc.scalar.dma_start(out=wkT_sb, in_=wkT_ext[:, :, :])
        nc.sync.dma_start(out=kT_sb[:, 0, :, :], in_=kT_ext[:, 0, :, :])
        nc.scalar.dma_start(out=qw_sb, in_=qw_ext[:, :, :])
        nc.sync.dma_start(out=kT_sb[:, 1, :, :], in_=kT_ext[:, 1, :, :])
        nc.scalar.dma_start(out=small_sb, in_=small_ext[:, :, :])
        nc.gpsimd.dma_start(out=mask_sb, in_=mask_ext[:, :])
        nc.vector.memset(ones_sb, 1.0)
        nc.vector.memset(onesk_sb, 1.0)
        nc.vector.memset(pihalf, PI / 2)
        nc.vector.memset(zero_b, 0.0)
        nc.vector.memset(shift_sb, -5.0)
        make_identity(nc, ident)
        nc.scalar.activation(dummy, pihalf, AF.Sin, scale=0.1)

        scores_a = scores_ps.tile([128, K // 2], F32, tag="sca")
        scores_b = scores_ps.tile([128, K // 2], F32, tag="scb")
        scores_c = [scores_a, scores_b]

        tick_ctx = ExitStack()
        tk_pool = tick_ctx.enter_context(
            tc.tile_pool(name="tk_ps", bufs=1, space="PSUM"))
        tk = tk_pool.tile([128, 128], F16)

        # ---- projections (kf first, c-outer: each kT column-half unblocks
        # its matmuls); per-(ht,c) kq casts on ScalarE ----
        setup_ctx = ExitStack()
        kf_ps_pool = setup_ctx.enter_context(
            tc.tile_pool(name="kf_ps", bufs=1, space="PSUM"))
        qf_ps_pool = setup_ctx.enter_context(
            tc.tile_pool(name="qf_ps", bufs=1, space="PSUM"))
        kf_ps = kf_ps_pool.tile([128, 2, K], F32, tag="kfp")
        for c in range(2):
            csl = slice(c * 512, (c + 1) * 512)
            for ht in range(2):
                hsl = slice(ht * 128, (ht + 1) * 128)
                nc.tensor.matmul(kf_ps[:, ht, csl], wkT_sb[:, 0, hsl],
                                 kT_sb[:, c, 0, :], start=True, stop=False)
                nc.tensor.matmul(kf_ps[:, ht, csl], wkT_sb[:, 1, hsl],
                                 kT_sb[:, c, 1, :], start=False, stop=True)
            # parallel casts: ht0 on ScalarE, ht1 on DVE
            nc.scalar.copy(kqk_sb[:, 0, c * 512:(c + 1) * 512],
                           kf_ps[:, 0, csl])
            nc.vector.tensor_copy(kqk_sb[:, 1, c * 512:(c + 1) * 512],
                                  kf_ps[:, 1, csl])

        ps_q = qf_ps_pool.tile([128, 2, Q], F32, tag="psq")
        for ht in range(2):
            hsl = slice(ht * 128, (ht + 1) * 128)
            nc.tensor.matmul(ps_q[:, ht, :], wqT_sb[:, 0, hsl], qT_sb[:, 0, :],
                             start=True, stop=False)
            nc.tensor.matmul(ps_q[:, ht, :], wqT_sb[:, 1, hsl], qT_sb[:, 1, :],
                             start=False, stop=True)
            nc.vector.tensor_copy(kqq_sb[:, ht, :], ps_q[:, ht, :])
        setup_ctx.close()

        # mask opener after the projections (mask DMA is on the slow queue)
        for c in range(2):
            csl = slice(c * 512, (c + 1) * 512)
            nc.tensor.matmul(scores_c[c], ones_sb, mask_sb[:, csl],
                             start=True, stop=False)

        # ---- DVE: zkq, then per-base fold chains in ACT order ----
        ts_abs(zkq_sb[:, :, 0:K], kqk_sb, 0.0, None, op0=OP.max, op1=None,
               patch="op0")
        ts_abs(zkq_sb[:, :, K:KQ], kqq_sb, 0.0, None, op0=OP.max, op1=None,
               patch="op0")
        # vals (512KB, needed only at the AV tail) issues only after zkq
        # exists, so it doesn't steal inbound DMA bandwidth from the
        # critical kT/weights path
        dma_gate = persist.tile([128, 1], F16)
        nc.gpsimd.tensor_copy(dma_gate, zkq_sb[:, 0, 0:1])
        nc.gpsimd.dma_start(out=val_sb, in_=vals_ext[:, :, :])

        def emit_kfolds(i):
            # every pass reads a FULL tile (sliced inputs drop the DVE to
            # 2x mode: measured 1353ns vs 753ns for the same element count)
            w = WBc[i]
            nf = _nfolds(w)
            if nf == 0:
                return None
            s_w = PI / (2 * w)
            C0 = 2 * PI if nf == 2 else PI
            asin = argk_pool.tile([128, 2, KQ], F16, tag="ka")
            a2 = argk_pool.tile([128, 2, 2, KQ], F16, tag="kb")
            a3 = argk_pool.tile([128, 2, 2, KQ], F16, tag="kc")
            a4 = argk_pool.tile([128, 2, 2, KQ], F16, tag="kd")
            ts_abs(asin[:, :, 0:K], kqk_sb, s_w, 0.0,
                   op0=OP.subtract, op1=OP.max, patch="op1")
            ts_abs(asin[:, :, K:KQ], kqq_sb, s_w, 0.0,
                   op0=OP.subtract, op1=OP.max, patch="op1")
            tick_deps.append(asin[:, 0, 0:128])
            nc.vector.tensor_scalar(a2[:, 0, :, :], asin, w, C0,
                                    op0=OP.mult, op1=OP.subtract)
            nc.vector.tensor_scalar(a2[:, 1, :, :], zkq_sb, w, C0,
                                    op0=OP.mult, op1=OP.subtract)
            tick_deps.append(a2[:, 1, 0, 0:128])
            if nf == 1:
                ts_abs(a3, a2, 0.0, PI / 2, op0=OP.max, op1=OP.subtract,
                       patch="op0")
                return a3
            ts_abs(a3, a2, 0.0, PI, op0=OP.max, op1=OP.subtract, patch="op0")
            tick_deps.append(a3[:, 1, 0, 0:128])
            ts_abs(a4, a3, 0.0, PI / 2, op0=OP.max, op1=OP.subtract,
                   patch="op0")
            return a4

        def emit_ksin(i, argk):
            if argk is None:
                w0_ = WBc[i]
                nc.scalar.activation(F_t[i][:, 0, :, 0:K], kqk_sb, AF.Sin,
                                     bias=zero_b, scale=w0_)
                nc.scalar.activation(F_t[i][:, 1, :, 0:K], kqk_sb, AF.Sin,
                                     bias=pihalf, scale=w0_)
                nc.scalar.activation(F_t[i][:, 0, :, K:KQ], kqq_sb, AF.Sin,
                                     bias=zero_b, scale=w0_)
                nc.scalar.activation(F_t[i][:, 1, :, K:KQ], kqq_sb, AF.Sin,
                                     bias=pihalf, scale=w0_)
            else:
                nc.scalar.activation(F_t[i], argk, AF.Sin, bias=zero_b,
                                     scale=1.0)

        def emit_g(j):
            # stationary build: g[:, j, gs] = F_j q-cols (side gs) * coef*wv
            for gs in range(2):
                sa, sb_ = broadcast_tensor_aps(F_t[j][:, gs, :, K:KQ],
                                               m2_slice(j, gs))
                nc.vector.tensor_tensor(out=g_sb[:, j, gs, :, :], in0=sa,
                                        in1=sb_, op=OP.mult)

        def emit_scd(d, bi):
            # scd[d][:,0] = sin*cos (q-cols); scd[d][:,1] = 1-2 sin^2
            nc.vector.tensor_tensor(out=scd[d][:, 0, :, :],
                                    in0=F_t[bi][:, 0, :, K:KQ],
                                    in1=F_t[bi][:, 1, :, K:KQ], op=OP.mult)
            nc.vector.tensor_tensor(out=s2tmp, in0=F_t[bi][:, 0, :, K:KQ],
                                    in1=F_t[bi][:, 0, :, K:KQ], op=OP.mult)
            nc.vector.tensor_scalar(scd[d][:, 1, :, :], s2tmp, -2.0, 1.0,
                                    op0=OP.mult, op1=OP.add)
            for kind in range(2):
                sa, sb_ = broadcast_tensor_aps(scd[d][:, kind, :, :],
                                               m2_slice(nb + d, kind))
                nc.vector.tensor_tensor(out=g_sb[:, nb + d, kind, :, :],
                                        in0=sa, in1=sb_, op=OP.mult)

        def emit_P(d, bi):
            # Pc first: the d-chunk gs0 matmuls consume it
            nc.vector.tensor_tensor(out=P_t[d][:, 1, :, :],
                                    in0=F_t[bi][:, 0, :, 0:K],
                                    in1=F_t[bi][:, 0, :, 0:K], op=OP.mult)
            nc.vector.tensor_tensor(out=P_t[d][:, 0, :, :],
                                    in0=F_t[bi][:, 0, :, 0:K],
                                    in1=F_t[bi][:, 1, :, 0:K], op=OP.mult)

        # ACT: sins in ORDER;  DVE: folds + per-base g/scd/P as sins land
        args = {}
        order_folded = [i for i in ORDER if _nfolds(WBc[i]) > 0]
        order_direct = [i for i in ORDER if _nfolds(WBc[i]) == 0]
        # fold chains first-two up-front, rest interleaved after g of direct
        for i in order_folded[:1]:
            args[i] = emit_kfolds(i)
        for i in order_direct:
            emit_ksin(i, None)
        for i in order_folded[:1]:
            emit_ksin(i, args[i])
        for i in order_direct:
            emit_g(i)
        prev = order_folded[0]
        for i in order_folded[1:]:
            args[i] = emit_kfolds(i)
            emit_ksin(i, args[i])
            # post-sin DVE work of the PREVIOUS folded base
            emit_g(prev)
            if prev in DBL:
                d = DBL.index(prev)
                emit_scd(d, prev)
                emit_P(d, prev)
            prev = i
        if cfg.get("quad"):
            # quad features from the double of the middle base: sin4q/4 =
            # sc*c2, cos4q = 1-8 sc^2; k-side Ps^2 / Ps*Pc products.
            squad = persist.tile([128, 2, 2, Q], F16)
            g_x = persist.tile([128, 2, Q], F16)
            P2q = persist.tile([128, 2, 2, K], F16)
            qslot = nb + nd
            nc.vector.tensor_tensor(out=squad[:, 0, :, :],
                                    in0=scd[0][:, 0, :, :],
                                    in1=scd[0][:, 1, :, :], op=OP.mult)
            nc.vector.tensor_tensor(out=s2tmp, in0=scd[0][:, 0, :, :],
                                    in1=scd[0][:, 0, :, :], op=OP.mult)
            nc.vector.tensor_scalar(squad[:, 1, :, :], s2tmp, -8.0, 1.0,
                                    op0=OP.mult, op1=OP.add)
            for kind in range(2):
                sa, sb_ = broadcast_tensor_aps(squad[:, kind, :, :],
                                               m2_slice(qslot, kind))
                nc.vector.tensor_tensor(out=g_sb[:, qslot, kind, :, :],
                                        in0=sa, in1=sb_, op=OP.mult)
            nc.vector.tensor_scalar_mul(g_x, g_sb[:, qslot, 1, :, :], -0.5)
            nc.vector.tensor_tensor(out=P2q[:, 1, :, :],
                                    in0=P_t[0][:, 0, :, :],
                                    in1=P_t[0][:, 0, :, :], op=OP.mult)
            nc.vector.tensor_tensor(out=P2q[:, 0, :, :],
                                    in0=P_t[0][:, 0, :, :],
                                    in1=P_t[0][:, 1, :, :], op=OP.mult)
            nc.vector.tensor_scalar_mul(u1_sb, scd[0][:, 0, :, :],
                                        float(2 * Ac[nb]))
            nc.vector.scalar_tensor_tensor(u2_sb, squad[:, 0, :, :],
                                           float(4 * Ac[nb + 1]), u1_sb,
                                           op0=OP.mult, op1=OP.add)
            ga, gb = broadcast_tensor_aps(u2_sb[:, :, :], wv2_sb[:, :, :])
            nc.vector.tensor_tensor(out=gone_sb, in0=ga, in1=gb, op=OP.mult)
            emit_g(prev)
        else:
            emit_g(prev)
            if prev in DBL:
                d = DBL.index(prev)
                emit_scd(d, prev)
            # ones-chunk combiners (need all scd)
            nc.vector.tensor_scalar_mul(u1_sb, scd[0][:, 0, :, :],
                                        float(2 * Ac[nb + 0]))
            nc.vector.scalar_tensor_tensor(u2_sb, scd[1][:, 0, :, :],
                                           float(2 * Ac[nb + 1]), u1_sb,
                                           op0=OP.mult, op1=OP.add)
            ga, gb = broadcast_tensor_aps(u2_sb[:, :, :], wv2_sb[:, :, :])
            nc.vector.tensor_tensor(out=gone_sb, in0=ga, in1=gb, op=OP.mult)
            if prev in DBL:
                emit_P(DBL.index(prev), prev)

        # ---- HAM ticklers bridging the feature ramp ----
        nc.tensor.transpose(tk, kqk_sb[:, 1, 0:128], ident)
        for dep in tick_deps[:6]:
            nc.tensor.transpose(tk, dep, ident)

        # ---- score chunks ----
        def emit_chunk(jslot, Fm, last=False):
            for gs in range(2):
                for t in range(2):
                    for c in range(2):
                        csl = slice(c * 512, (c + 1) * 512)
                        stop = last and gs == 1 and t == 1
                        nc.tensor.matmul(scores_c[c], g_sb[:, jslot, gs, t, :],
                                         Fm[:, 1 - gs, t, csl],
                                         start=False, stop=stop)

        chunk_order = []
        for i in ORDER:
            chunk_order.append(("b", i))
            if i in DBL:
                chunk_order.append(("d", DBL.index(i)))
        if cfg.get("quad"):
            # [b0, b1, d0, quad, ones, b2]: the last base carries no products
            chunk_order.insert(3, ("q4", None))
            chunk_order.insert(4, ("ones", None))
        else:
            # ones inserted before the final double
            chunk_order.insert(len(chunk_order) - 1, ("ones", None))
        for idx, (kind, v) in enumerate(chunk_order):
            last = idx == len(chunk_order) - 1
            if kind == "b":
                emit_chunk(v, F_t[v], last=last)
            elif kind == "d":
                emit_chunk(nb + v, P_t[v], last=last)
            elif kind == "q4":
                for stat, mov in ((g_sb[:, nb + nd, 0, :, :], P2q[:, 1, :, :]),
                                  (g_sb[:, nb + nd, 1, :, :], P2q[:, 0, :, :]),
                                  (g_x, P_t[0][:, 0, :, :])):
                    for t in range(2):
                        for c in range(2):
                            csl = slice(c * 512, (c + 1) * 512)
                            nc.tensor.matmul(scores_c[c], stat[:, t, :],
                                             mov[:, t, csl],
                                             start=False, stop=False)
            else:
                for t in range(2):
                    for c in range(2):
                        nc.tensor.matmul(scores_c[c], gone_sb[:, t, :],
                                         onesk_sb, start=False, stop=False)
        tick_ctx.close()

        # ---- softmax: exp(s-5), row sums on DVE (idle by then) ----
        for qtr in range(4):
            sc = scores_c[qtr // 2]
            off = (qtr % 2) * 256
            nc.scalar.activation(E_q[qtr], sc[:, off:off + 256], AF.Exp,
                                 bias=shift_sb, scale=1.0)
            nc.vector.tensor_reduce(rs_q[qtr], E_q[qtr],
                                    axis=mybir.AxisListType.X,
                                    op=mybir.AluOpType.add)
        nc.vector.tensor_add(rowsum, rs_q[0], rs_q[1])
        nc.vector.tensor_add(rs_t, rs_q[2], rs_q[3])
        nc.vector.tensor_add(rowsum, rowsum, rs_t)
        nc.vector.reciprocal(rinv, rowsum)

        # ---- attn @ values ----
        with ExitStack() as tail_ctx:
            tp_ps = tail_ctx.enter_context(
                tc.tile_pool(name="tp_ps", bufs=2, space="PSUM"))
            av_ps = tail_ctx.enter_context(
                tc.tile_pool(name="av_ps", bufs=1, space="PSUM"))
            for kt in range(8):
                E_src = E_q[kt // 2]
                off = (kt % 2) * 128
                tp = tp_ps.tile([128, 128], F16, tag="tp")
                nc.tensor.transpose(tp, E_src[:, off:off + 128], ident)
                nc.vector.tensor_copy(ET_sb[:, kt, :], tp)
            ps_av = av_ps.tile([Q, D], F32)
            for kt in range(8):
                nc.tensor.matmul(ps_av, ET_sb[:, kt, :], val_sb[:, kt, :],
                                 start=(kt == 0), stop=(kt == 7))
            nc.vector.tensor_scalar_mul(out_sb[:, 0:128], ps_av[:, 0:128], rinv)
            nc.sync.dma_start(out=out_ext[:, 0:128], in_=out_sb[:, 0:128])
            nc.vector.tensor_scalar_mul(out_sb[:, 128:256], ps_av[:, 128:256], rinv)
        nc.sync.dma_start(out=out_ext[:, 128:256], in_=out_sb[:, 128:256])

    _patch_bir(nc, abs_patch)
    return nc


def _patch_bir(nc, abs_patch):
    import json

    d = json.loads(nc.to_json_bytes())
    patch_map = dict(abs_patch)
    k = [0]
    self_drop = {"Activation": "Activation", "DVE": "DVE", "Pool": "Pool"}
    compute_ops = {"Activation", "TensorScalarPtr", "TensorScalar", "TensorTensor",
                   "TensorCopy", "TensorReduce", "Reciprocal", "Memset"}
    n_abs = 0
    for fn in d["functions"]:
        for blk in fn["blocks"]:
            out = []
            for inst in blk["instructions"]:
                slot = patch_map.get(inst.get("name"))
                if slot is not None:
                    inst[slot] = "abs"
                    n_abs += 1
                si = inst.get("sync_info") or {}
                ow = si.get("on_wait") or []
                op = inst.get("opcode")
                eng = inst.get("engine")
                if len(ow) > 1 and op != "EventSemaphore":
                    if op in compute_ops and eng in self_drop:
                        pref = self_drop[eng] + "_"
                        ow = [w for w in ow
                              if not str(w.get("ant_name", "")).startswith(pref)]
                    while len(ow) > 1:
                        w = ow.pop(0)
                        k[0] += 1
                        out.append({
                            "debug": inst.get("debug", 0), "engine": eng,
                            "ins": [], "name": f"WSplit-{k[0]}",
                            "opcode": "EventSemaphore", "outs": [],
                            "sync_info": {"on_update": [], "on_wait": [w]},
                        })
                    si["on_wait"] = ow
                out.append(inst)
            blk["instructions"] = out
    assert n_abs == len(abs_patch), (n_abs, len(abs_patch))
    patched = json.dumps(d).encode()
    nc.to_json_bytes = lambda: patched


def _get_nc():
    if "nc" not in _CACHE:
        _CACHE["nc"] = _build_bass(CONFIG)
    return _CACHE["nc"]


def _host_prep(queries, keys, values, W_q, W_k, w_v, valid_lens):
    cfg = CONFIG
    nb = len(cfg["wb"])
    nd = len(cfg["dbl"])
    nst = nb + nd + (1 if cfg.get("quad") else 0)
    Ac = cfg["a"]

    queries = np.asarray(queries, dtype=np.float32)
    keys = np.asarray(keys, dtype=np.float32)
    values = np.asarray(values, dtype=np.float32)
    W_q = np.asarray(W_q, dtype=np.float32)
    W_k = np.asarray(W_k, dtype=np.float32)
    w_v = np.asarray(w_v, dtype=np.float32)
    valid = np.asarray(valid_lens).astype(np.int64)

    def part2(x):
        # [2*128, N] -> [128, 2, N]
        return x.reshape(2, 128, -1).transpose(1, 0, 2)

    wqT = part2(W_q.T.astype(np.float16))            # [128, 2, H]
    wkT = part2(W_k.T.astype(np.float16))
    wv2 = w_v.reshape(2, 128).T                      # (p, t)
    coef = np.zeros((nst, 2), np.float32)
    for j in range(nb):
        coef[j] = (Ac[j], Ac[j])
    for d in range(nd):
        # scd kind0 = sin*cos = sin(2w)/2 -> Pc-pair coef -4a; kind1 = cos2w -> +2a
        coef[nb + d] = (-4 * Ac[nb + d], 2 * Ac[nb + d])
    if cfg.get("quad"):
        # squad kind0 = sin4q/4 -> -8a*4; kind1 = cos4q -> -8a
        coef[nb + nd] = (-32 * Ac[nb + nd], -8 * Ac[nb + nd])
    # M2 block [p, t, j*2+gs]
    M2t = (wv2[:, :, None, None] * coef[None, None, :, :]).reshape(128, 2, -1)
    wv2_t = wv2[:, :, None]

    kidx = np.arange(K)
    in_maps = []
    for b in range(B):
        mask = np.where(kidx < valid[b], np.float16(0.0), np.float16(NEG_BIG))
        qTc = part2(queries[b].T.astype(np.float16))     # [128, 2, Q]
        qw = np.concatenate([qTc, wqT], axis=2)
        small = np.concatenate(
            [M2t.astype(np.float16), wv2_t.astype(np.float16)], axis=2)
        in_maps.append({
            "kT4": np.ascontiguousarray(part2(keys[b].T.astype(np.float16))
                .reshape(128, 2, 2, 512).transpose(0, 2, 1, 3)),
            "vals3": np.ascontiguousarray(
                values[b].astype(np.float16).reshape(8, 128, D)
                .transpose(1, 0, 2)),
            "wkTp": np.ascontiguousarray(wkT),
            "qw": np.ascontiguousarray(qw),
            "small": np.ascontiguousarray(small),
            "mask": np.ascontiguousarray(mask.reshape(1, K)),
        })
    return in_maps, valid, values


def _run(inputs, trace=False, **kw):
    from concourse.bass_utils import run_bass_kernel_spmd

    nc = _get_nc()
    in_maps, valid, values = _host_prep(**inputs)
    res = run_bass_kernel_spmd(nc, in_maps, list(range(B)), trace=trace, **kw)
    out = np.stack([np.asarray(res.results[i]["out"], dtype=np.float32)
                    for i in range(B)])
    for b in range(B):
        if valid[b] == 0:
            out[b] = np.broadcast_to(values[b].mean(axis=0), (Q, D))
    return out, res


def kernel(**inputs):
    out, _ = _run(inputs, trace=False)
    return out
